# revision 5
# baseline (speedup 1.0000x reference)
"""K-means step kernel for Trainium2 (8 NeuronCores, data-parallel over n).

scores[n,k] = ||c_k||^2 - 2 x_n.c_k ; assign = argmin_k ; new centroids =
segment-mean.  Strategy per core (n_loc = n/8 rows):
  mm1: dot' = x @ (-2C)^T via 3-term fp16 split (x_hi*C_hi + x_hi*C_lo +
       x_lo*C_hi) accumulated in fp32 PSUM -> fp32-accurate scores at
       1 cyc/row instead of fp32 matmul's 4.
  DVE: tensor_tensor_reduce fuses (dot' + c_sq) with a running min ->
       scores in SBUF + per-row min; tensor_scalar is_equal -> one-hot (f16).
  mm2: partial_sums = onehot^T @ [x_hi, 1] + onehot^T @ [x_lo, 0]
       (exact fp32 sums + exact counts in the appended column).
  Host: sum the 8 per-core partials, divide, keep old centroid where empty.
"""

import numpy as np

import concourse.bass as bass
import concourse.mybir as mybir
import concourse.tile as tile
from concourse.bass_utils import run_bass_kernel_spmd
from concourse.vector_clock import ScopedClock

# ---------------------------------------------------------------------------
# Workaround: walrus rejects >1 sem wait on CTRL (drain/nop) instructions.
# Split the TileContext exit-drain's waits across one NOP per wait.
_MAXW = 1


def _patched_drain_and_barrier(self, tick_clock, wait_clock):
    nc = self.nc
    drain_inst = nc.sync.drain()
    wait_clock.add_sem_waits(
        drain_inst.ins, ScopedClock({None: tick_clock.global_clock})
    )
    si = drain_inst.ins.sync_info
    waits = list(si.on_wait) if si and si.on_wait else []
    if len(waits) > _MAXW:
        drain_inst.ins.sync_info = mybir.SyncInfo(
            on_wait=waits[:_MAXW], on_update=list(si.on_update or [])
        )
        rest = waits[_MAXW:]
        for i in range(0, len(rest), _MAXW):
            nop = nc.sync.nop()
            nop.ins.sync_info = mybir.SyncInfo(
                on_wait=rest[i : i + _MAXW], on_update=[]
            )
    nc.all_engine_barrier()
    popped = nc._tile_sem_poison_stack.pop()
    assert popped is self._sem_poison
    nc.clear_and_free_semaphores(list(self.sems.allocated().values()))
    nc.all_engine_barrier()


tile.TileContext._drain_and_barrier = _patched_drain_and_barrier

# This walrus build accepts only ONE sync wait per instruction, but Tile's
# scheduler emits several on phase joins.  Rewrite the BIR before compiling:
# excess waits move onto same-engine NOPs inserted just before the
# instruction (identical semantics: all waits still complete before it).
import json as _json

import concourse.bass2jax as _bass2jax

_orig_compile_bir = _bass2jax.compile_bir_kernel


def _split_waits_compile(bir_json, tmpdir, neff_name="file.neff"):
    j = _json.loads(bir_json)
    cnt = 0
    for f in j["functions"]:
        for bb in f["blocks"]:
            out = []
            for ins in bb["instructions"]:
                si = ins.get("sync_info")
                ow = (si or {}).get("on_wait") or []
                if len(ow) > 1:
                    for w in ow[:-1]:
                        cnt += 1
                        out.append(
                            {
                                "debug": ins.get("debug"),
                                "engine": ins["engine"],
                                "ins": [],
                                "outs": [],
                                "name": f"I-wsplit-{cnt}",
                                "opcode": "NoOp",
                                "sync_info": {"on_update": [], "on_wait": [w]},
                            }
                        )
                    si["on_wait"] = [ow[-1]]
                out.append(ins)
            bb["instructions"] = out
    return _orig_compile_bir(_json.dumps(j).encode(), tmpdir, neff_name=neff_name)


_bass2jax.compile_bir_kernel = _split_waits_compile
# ---------------------------------------------------------------------------

N_CORES = 8
P = 128
F16 = mybir.dt.float16
F32 = mybir.dt.float32
ADD = mybir.AluOpType.add
MIN = mybir.AluOpType.min
EQ = mybir.AluOpType.is_equal

_KERNEL_CACHE = {}


def build_kernel(n_loc, k, d, group=8):
    ntiles = n_loc // P
    ndh = d // P            # 128-row halves of the contraction dim
    nq = k // 512           # 512-wide k quarters for mm1 (1 PSUM bank each)
    nchunks = k // P        # 128-row output chunks for mm2
    daug = d + 1

    nc = bass.Bass()
    xT_hi = nc.declare_dram_parameter("xT_hi", [d, n_loc], F16, isOutput=False)
    xT_lo = nc.declare_dram_parameter("xT_lo", [d, n_loc], F16, isOutput=False)
    xa_hi = nc.declare_dram_parameter("xa_hi", [n_loc, daug], F16, isOutput=False)
    xa_lo = nc.declare_dram_parameter("xa_lo", [n_loc, daug], F16, isOutput=False)
    ChiT = nc.declare_dram_parameter("ChiT", [d, k], F16, isOutput=False)
    CloT = nc.declare_dram_parameter("CloT", [d, k], F16, isOutput=False)
    csqb = nc.declare_dram_parameter("csqb", [P, k], F32, isOutput=False)
    out = nc.declare_dram_parameter("out", [k, daug], F32, isOutput=True)

    with tile.TileContext(nc) as tc:
        with (
            tc.tile_pool(name="consts", bufs=1) as consts,
            tc.tile_pool(name="xt", bufs=3) as xtp,
            tc.tile_pool(name="xaug", bufs=2 * group + 2) as xap,
            tc.tile_pool(name="oh", bufs=2 * group + 2) as ohp,
            tc.tile_pool(name="sc", bufs=3) as scp,
            tc.tile_pool(name="mp", bufs=6) as mp,
            tc.tile_pool(name="ps1", bufs=3, space="PSUM") as ps1,
            tc.tile_pool(name="ps2", bufs=2, space="PSUM") as ps2,
        ):
            chi = [consts.tile([P, k], F16, tag=f"chi{i}", name=f"chi{i}") for i in range(ndh)]
            clo = [consts.tile([P, k], F16, tag=f"clo{i}", name=f"clo{i}") for i in range(ndh)]
            for i in range(ndh):
                nc.sync.dma_start(out=chi[i], in_=ChiT[i * P : (i + 1) * P, :])
                nc.sync.dma_start(out=clo[i], in_=CloT[i * P : (i + 1) * P, :])
            csq = consts.tile([P, k], F32, tag="csq", name="csq")
            nc.sync.dma_start(out=csq, in_=csqb[:, :])
            acc = consts.tile([P, nchunks * daug], F32, tag="acc", name="acc")
            nc.vector.memset(acc, 0.0)

            def emit_mm2(pend):
                ohs, xhis, xlos = pend
                ng = len(ohs)
                for c in range(nchunks):
                    pc = ps2.tile([P, daug], F32, tag="ps2", name="pc")
                    for g in range(ng):
                        nc.tensor.matmul(
                            pc, ohs[g][:, c * P : (c + 1) * P], xhis[g],
                            start=(g == 0), stop=False,
                        )
                    for g in range(ng):
                        nc.tensor.matmul(
                            pc, ohs[g][:, c * P : (c + 1) * P], xlos[g],
                            start=False, stop=(g == ng - 1),
                        )
                    nc.vector.tensor_tensor(
                        acc[:, c * daug : (c + 1) * daug], pc,
                        acc[:, c * daug : (c + 1) * daug], op=ADD,
                    )

            pending = None
            cur = ([], [], [])
            for i in range(ntiles):
                xth = [xtp.tile([P, P], F16, tag=f"xth{j}", name=f"xth{j}") for j in range(ndh)]
                xtl = [xtp.tile([P, P], F16, tag=f"xtl{j}", name=f"xtl{j}") for j in range(ndh)]
                for j in range(ndh):
                    nc.sync.dma_start(
                        out=xth[j], in_=xT_hi[j * P : (j + 1) * P, i * P : (i + 1) * P]
                    )
                    nc.sync.dma_start(
                        out=xtl[j], in_=xT_lo[j * P : (j + 1) * P, i * P : (i + 1) * P]
                    )
                xh = xap.tile([P, daug], F16, tag="xah", name="xah")
                xl = xap.tile([P, daug], F16, tag="xal", name="xal")
                nc.sync.dma_start(out=xh, in_=xa_hi[i * P : (i + 1) * P, :])
                nc.sync.dma_start(out=xl, in_=xa_lo[i * P : (i + 1) * P, :])

                scores = scp.tile([P, k], F32, tag="scores", name="scores")
                m_prev = None
                for h in range(nq // 2):  # 1024-wide halves (2 banks PSUM)
                    ph = ps1.tile([P, 1024], F32, tag="ps1", name="ph")
                    for q in range(2):  # 512-wide accumulation groups
                        col = h * 1024 + q * 512
                        terms = []
                        for j in range(ndh):
                            terms.append((xth[j], chi[j]))
                            terms.append((xth[j], clo[j]))
                        for j in range(ndh):
                            terms.append((xtl[j], chi[j]))
                        for t, (w, cm) in enumerate(terms):
                            nc.tensor.matmul(
                                ph[:, q * 512 : (q + 1) * 512],
                                w, cm[:, col : col + 512],
                                start=(t == 0), stop=(t == len(terms) - 1),
                            )
                    mh = mp.tile([P, 1], F32, tag=f"m{h % 2}", name=f"mh{h % 2}")
                    nc.vector.tensor_tensor(
                        scores[:, h * 1024 : (h + 1) * 1024],
                        ph, csq[:, h * 1024 : (h + 1) * 1024], op=ADD,
                    )
                    nc.vector.tensor_reduce(
                        mh, scores[:, h * 1024 : (h + 1) * 1024],
                        axis=mybir.AxisListType.X, op=MIN,
                    )
                    if m_prev is not None:
                        m2 = mp.tile([P, 1], F32, tag="mfin", name="m2")
                        nc.vector.tensor_tensor(m2, mh, m_prev, op=MIN)
                        mh = m2
                    m_prev = mh
                oh_t = ohp.tile([P, k], F16, tag="oh", name="oh_t")
                nc.vector.tensor_scalar(
                    out=oh_t, in0=scores, scalar1=m_prev, scalar2=None, op0=EQ
                )
                cur[0].append(oh_t)
                cur[1].append(xh)
                cur[2].append(xl)

                if len(cur[0]) == group:
                    if pending is not None:
                        emit_mm2(pending)
                    pending = cur
                    cur = ([], [], [])
            if pending is not None:
                emit_mm2(pending)
            if cur[0]:
                emit_mm2(cur)

            for c in range(nchunks):
                nc.sync.dma_start(
                    out=out[c * P : (c + 1) * P, :],
                    in_=acc[:, c * daug : (c + 1) * daug],
                )
    return nc


def _prep_inputs(x, C):
    n, d = x.shape
    k = C.shape[0]
    n_loc = n // N_CORES

    Cp = -2.0 * C.astype(np.float64)
    c_sq = np.sum(C.astype(np.float64) ** 2, axis=1).astype(np.float32)
    Chi = Cp.astype(np.float16)
    Clo = (Cp - Chi.astype(np.float64)).astype(np.float16)
    ChiT = np.ascontiguousarray(Chi.T)
    CloT = np.ascontiguousarray(Clo.T)
    csqb = np.ascontiguousarray(np.broadcast_to(c_sq, (P, k)))

    xh = x.astype(np.float16)
    xl = (x.astype(np.float64) - xh.astype(np.float64)).astype(np.float16)
    ones = np.ones((n_loc, 1), np.float16)
    zeros = np.zeros((n_loc, 1), np.float16)

    in_maps = []
    for c in range(N_CORES):
        sl = slice(c * n_loc, (c + 1) * n_loc)
        in_maps.append(
            {
                "xT_hi": np.ascontiguousarray(xh[sl].T),
                "xT_lo": np.ascontiguousarray(xl[sl].T),
                "xa_hi": np.ascontiguousarray(np.concatenate([xh[sl], ones], 1)),
                "xa_lo": np.ascontiguousarray(np.concatenate([xl[sl], zeros], 1)),
                "ChiT": ChiT,
                "CloT": CloT,
                "csqb": csqb,
            }
        )
    return in_maps


def kernel(x, centroids, _trace=False):
    x = np.asarray(x, dtype=np.float32)
    C = np.asarray(centroids, dtype=np.float32)
    n, d = x.shape
    k = C.shape[0]
    n_loc = n // N_CORES

    key = (n_loc, k, d)
    if key not in _KERNEL_CACHE:
        _KERNEL_CACHE[key] = build_kernel(n_loc, k, d)
    nc = _KERNEL_CACHE[key]

    in_maps = _prep_inputs(x, C)
    res = run_bass_kernel_spmd(
        nc, in_maps, core_ids=list(range(N_CORES)), trace=_trace
    )

    total = np.zeros((k, d + 1), np.float64)
    for c in range(N_CORES):
        total += res.results[c]["out"].astype(np.float64)
    sums = total[:, :d]
    counts = total[:, d]
    means = (sums / np.maximum(counts, 1.0)[:, None]).astype(np.float32)
    out = np.where(counts[:, None] > 0, means, C)
    if _trace:
        kernel._last_result = res
    return out.astype(np.float32)


# revision 8
# speedup vs baseline: 8529.6531x; 8529.6531x over previous
"""K-means step kernel for Trainium2 (8 NeuronCores, data-parallel over n).

scores[n,k] = ||c_k||^2 - 2 x_n.c_k ; assign = argmin_k ; new centroids =
segment-mean.  Strategy per core (n_loc = n/8 rows):
  mm1: dot' = x @ (-2C)^T via 3-term fp16 split (x_hi*C_hi + x_hi*C_lo +
       x_lo*C_hi) accumulated in fp32 PSUM -> fp32-accurate scores at
       1 cyc/row instead of fp32 matmul's 4.
  DVE: tensor_tensor_reduce fuses (dot' + c_sq) with a running min ->
       scores in SBUF + per-row min; tensor_scalar is_equal -> one-hot (f16).
  mm2: partial_sums = onehot^T @ [x_hi, 1] + onehot^T @ [x_lo, 0]
       (exact fp32 sums + exact counts in the appended column).
  Host: sum the 8 per-core partials, divide, keep old centroid where empty.
"""

import numpy as np

import concourse.bass as bass
import concourse.mybir as mybir
import concourse.tile as tile
from concourse.bass_utils import run_bass_kernel_spmd
from concourse.vector_clock import ScopedClock

# ---------------------------------------------------------------------------
# Workaround: walrus rejects >1 sem wait on CTRL (drain/nop) instructions.
# Split the TileContext exit-drain's waits across one NOP per wait.
_MAXW = 1


def _patched_drain_and_barrier(self, tick_clock, wait_clock):
    nc = self.nc
    drain_inst = nc.sync.drain()
    wait_clock.add_sem_waits(
        drain_inst.ins, ScopedClock({None: tick_clock.global_clock})
    )
    si = drain_inst.ins.sync_info
    waits = list(si.on_wait) if si and si.on_wait else []
    if len(waits) > _MAXW:
        drain_inst.ins.sync_info = mybir.SyncInfo(
            on_wait=waits[:_MAXW], on_update=list(si.on_update or [])
        )
        rest = waits[_MAXW:]
        for i in range(0, len(rest), _MAXW):
            nop = nc.sync.nop()
            nop.ins.sync_info = mybir.SyncInfo(
                on_wait=rest[i : i + _MAXW], on_update=[]
            )
    nc.all_engine_barrier()
    popped = nc._tile_sem_poison_stack.pop()
    assert popped is self._sem_poison
    nc.clear_and_free_semaphores(list(self.sems.allocated().values()))
    nc.all_engine_barrier()


tile.TileContext._drain_and_barrier = _patched_drain_and_barrier

# This walrus build accepts only ONE sync wait per instruction, but Tile's
# scheduler emits several on phase joins.  Rewrite the BIR before compiling:
# excess waits move onto same-engine NOPs inserted just before the
# instruction (identical semantics: all waits still complete before it).
import json as _json

import concourse.bass2jax as _bass2jax

_orig_compile_bir = _bass2jax.compile_bir_kernel


def _split_waits_compile(bir_json, tmpdir, neff_name="file.neff"):
    j = _json.loads(bir_json)
    cnt = 0
    for f in j["functions"]:
        for bb in f["blocks"]:
            out = []
            for ins in bb["instructions"]:
                si = ins.get("sync_info")
                ow = (si or {}).get("on_wait") or []
                if len(ow) > 1:
                    for w in ow[:-1]:
                        cnt += 1
                        out.append(
                            {
                                "debug": ins.get("debug"),
                                "engine": ins["engine"],
                                "ins": [],
                                "outs": [],
                                "name": f"I-wsplit-{cnt}",
                                "opcode": "NoOp",
                                "sync_info": {"on_update": [], "on_wait": [w]},
                            }
                        )
                    si["on_wait"] = [ow[-1]]
                out.append(ins)
            bb["instructions"] = out
    return _orig_compile_bir(_json.dumps(j).encode(), tmpdir, neff_name=neff_name)


_bass2jax.compile_bir_kernel = _split_waits_compile
# ---------------------------------------------------------------------------

N_CORES = 8
P = 128
F16 = mybir.dt.float16
F32 = mybir.dt.float32
ADD = mybir.AluOpType.add
MIN = mybir.AluOpType.min
EQ = mybir.AluOpType.is_equal

_KERNEL_CACHE = {}


def build_kernel(n_loc, k, d, group=8, ps1_bufs=3, ps2_bufs=2, xt_bufs=3, sc_bufs=3, exact_mm2=True):
    ntiles = n_loc // P
    ndh = d // P            # 128-row halves of the contraction dim
    nq = k // 512           # 512-wide k quarters for mm1 (1 PSUM bank each)
    nchunks = k // P        # 128-row output chunks for mm2
    daug = d + 1

    nc = bass.Bass()
    xT_hi = nc.declare_dram_parameter("xT_hi", [d, n_loc], F16, isOutput=False)
    xT_lo = nc.declare_dram_parameter("xT_lo", [d, n_loc], F16, isOutput=False)
    xa_hi = nc.declare_dram_parameter("xa_hi", [n_loc, daug], F16, isOutput=False)
    xa_lo = nc.declare_dram_parameter("xa_lo", [n_loc, daug], F16, isOutput=False)
    ChiT = nc.declare_dram_parameter("ChiT", [d, k], F16, isOutput=False)
    CloT = nc.declare_dram_parameter("CloT", [d, k], F16, isOutput=False)
    csqb = nc.declare_dram_parameter("csqb", [P, k], F32, isOutput=False)
    out = nc.declare_dram_parameter("out", [k, daug], F32, isOutput=True)

    with tile.TileContext(nc) as tc:
        with (
            tc.tile_pool(name="consts", bufs=1) as consts,
            tc.tile_pool(name="xt", bufs=xt_bufs) as xtp,
            tc.tile_pool(name="xaug", bufs=2 * group + 2) as xap,
            tc.tile_pool(name="oh", bufs=2 * group + 2) as ohp,
            tc.tile_pool(name="sc", bufs=sc_bufs) as scp,
            tc.tile_pool(name="mp", bufs=6) as mp,
            tc.tile_pool(name="ps1", bufs=ps1_bufs, space="PSUM") as ps1,
            tc.tile_pool(name="ps2", bufs=ps2_bufs, space="PSUM") as ps2,
        ):
            chi = [consts.tile([P, k], F16, tag=f"chi{i}", name=f"chi{i}") for i in range(ndh)]
            clo = [consts.tile([P, k], F16, tag=f"clo{i}", name=f"clo{i}") for i in range(ndh)]
            for i in range(ndh):
                nc.sync.dma_start(out=chi[i], in_=ChiT[i * P : (i + 1) * P, :])
                nc.sync.dma_start(out=clo[i], in_=CloT[i * P : (i + 1) * P, :])
            csq = consts.tile([P, k], F32, tag="csq", name="csq")
            nc.sync.dma_start(out=csq, in_=csqb[:, :])
            acc = consts.tile([P, nchunks * daug], F32, tag="acc", name="acc")
            nc.vector.memset(acc, 0.0)

            def emit_mm2(pend):
                ohs, xhis, xlos = pend
                ng = len(ohs)
                for c in range(nchunks):
                    pc = ps2.tile([P, daug], F32, tag="ps2", name="pc")
                    for g in range(ng):
                        nc.tensor.matmul(
                            pc, ohs[g][:, c * P : (c + 1) * P], xhis[g],
                            start=(g == 0),
                            stop=(not exact_mm2 and g == ng - 1),
                        )
                    if exact_mm2:
                        for g in range(ng):
                            nc.tensor.matmul(
                                pc, ohs[g][:, c * P : (c + 1) * P], xlos[g],
                                start=False, stop=(g == ng - 1),
                            )
                    nc.vector.tensor_tensor(
                        acc[:, c * daug : (c + 1) * daug], pc,
                        acc[:, c * daug : (c + 1) * daug], op=ADD,
                    )

            pending = None
            cur = ([], [], [])
            for i in range(ntiles):
                xth = [xtp.tile([P, P], F16, tag=f"xth{j}", name=f"xth{j}") for j in range(ndh)]
                xtl = [xtp.tile([P, P], F16, tag=f"xtl{j}", name=f"xtl{j}") for j in range(ndh)]
                for j in range(ndh):
                    nc.sync.dma_start(
                        out=xth[j], in_=xT_hi[j * P : (j + 1) * P, i * P : (i + 1) * P]
                    )
                    nc.sync.dma_start(
                        out=xtl[j], in_=xT_lo[j * P : (j + 1) * P, i * P : (i + 1) * P]
                    )
                xh = xap.tile([P, daug], F16, tag="xah", name="xah")
                xl = xap.tile([P, daug], F16, tag="xal", name="xal")
                nc.sync.dma_start(out=xh, in_=xa_hi[i * P : (i + 1) * P, :])
                nc.sync.dma_start(out=xl, in_=xa_lo[i * P : (i + 1) * P, :])

                scores = scp.tile([P, k], F32, tag="scores", name="scores")
                m_prev = None
                for h in range(nq // 2):  # 1024-wide halves (2 banks PSUM)
                    ph = ps1.tile([P, 1024], F32, tag="ps1", name="ph")
                    for q in range(2):  # 512-wide accumulation groups
                        col = h * 1024 + q * 512
                        terms = []
                        for j in range(ndh):
                            terms.append((xth[j], chi[j]))
                            terms.append((xth[j], clo[j]))
                        for j in range(ndh):
                            terms.append((xtl[j], chi[j]))
                        for t, (w, cm) in enumerate(terms):
                            nc.tensor.matmul(
                                ph[:, q * 512 : (q + 1) * 512],
                                w, cm[:, col : col + 512],
                                start=(t == 0), stop=(t == len(terms) - 1),
                            )
                    mh = mp.tile([P, 1], F32, tag=f"m{h % 2}", name=f"mh{h % 2}")
                    nc.vector.tensor_tensor(
                        scores[:, h * 1024 : (h + 1) * 1024],
                        ph, csq[:, h * 1024 : (h + 1) * 1024], op=ADD,
                    )
                    nc.vector.tensor_reduce(
                        mh, scores[:, h * 1024 : (h + 1) * 1024],
                        axis=mybir.AxisListType.X, op=MIN,
                    )
                    if m_prev is not None:
                        m2 = mp.tile([P, 1], F32, tag="mfin", name="m2")
                        nc.vector.tensor_tensor(m2, mh, m_prev, op=MIN)
                        mh = m2
                    m_prev = mh
                oh_t = ohp.tile([P, k], F16, tag="oh", name="oh_t")
                nc.vector.tensor_scalar(
                    out=oh_t, in0=scores, scalar1=m_prev, scalar2=None, op0=EQ
                )
                cur[0].append(oh_t)
                cur[1].append(xh)
                cur[2].append(xl)

                if len(cur[0]) == group:
                    if pending is not None:
                        emit_mm2(pending)
                    pending = cur
                    cur = ([], [], [])
            if pending is not None:
                emit_mm2(pending)
            if cur[0]:
                emit_mm2(cur)

            for c in range(nchunks):
                nc.sync.dma_start(
                    out=out[c * P : (c + 1) * P, :],
                    in_=acc[:, c * daug : (c + 1) * daug],
                )
    return nc


def _prep_inputs(x, C):
    n, d = x.shape
    k = C.shape[0]
    n_loc = n // N_CORES

    Cp = -2.0 * C.astype(np.float64)
    c_sq = np.sum(C.astype(np.float64) ** 2, axis=1).astype(np.float32)
    Chi = Cp.astype(np.float16)
    Clo = (Cp - Chi.astype(np.float64)).astype(np.float16)
    ChiT = np.ascontiguousarray(Chi.T)
    CloT = np.ascontiguousarray(Clo.T)
    csqb = np.ascontiguousarray(np.broadcast_to(c_sq, (P, k)))

    xh = x.astype(np.float16)
    xl = (x.astype(np.float64) - xh.astype(np.float64)).astype(np.float16)
    ones = np.ones((n_loc, 1), np.float16)
    zeros = np.zeros((n_loc, 1), np.float16)

    in_maps = []
    for c in range(N_CORES):
        sl = slice(c * n_loc, (c + 1) * n_loc)
        in_maps.append(
            {
                "xT_hi": np.ascontiguousarray(xh[sl].T),
                "xT_lo": np.ascontiguousarray(xl[sl].T),
                "xa_hi": np.ascontiguousarray(np.concatenate([xh[sl], ones], 1)),
                "xa_lo": np.ascontiguousarray(np.concatenate([xl[sl], zeros], 1)),
                "ChiT": ChiT,
                "CloT": CloT,
                "csqb": csqb,
            }
        )
    return in_maps


def kernel(x, centroids, _trace=False):
    x = np.asarray(x, dtype=np.float32)
    C = np.asarray(centroids, dtype=np.float32)
    n, d = x.shape
    k = C.shape[0]
    n_loc = n // N_CORES

    key = (n_loc, k, d)
    if key not in _KERNEL_CACHE:
        _KERNEL_CACHE[key] = build_kernel(n_loc, k, d)
    nc = _KERNEL_CACHE[key]

    in_maps = _prep_inputs(x, C)
    res = run_bass_kernel_spmd(
        nc, in_maps, core_ids=list(range(N_CORES)), trace=_trace
    )

    total = np.zeros((k, d + 1), np.float64)
    for c in range(N_CORES):
        total += res.results[c]["out"].astype(np.float64)
    sums = total[:, :d]
    counts = total[:, d]
    means = (sums / np.maximum(counts, 1.0)[:, None]).astype(np.float32)
    out = np.where(counts[:, None] > 0, means, C)
    if _trace:
        kernel._last_result = res
    return out.astype(np.float32)
